# revision 7
# baseline (speedup 1.0000x reference)
"""V4: 16-bit time-split CRF forward kernel for Trainium2.

Structure (generalizing the V3 time-split):
- 8S chains total: each of 8 cores runs S independent sub-chains, each
  covering chunk = 64/S consecutive time steps plus W warmup steps from
  uniform init and one handoff step (NSTEP = chunk + W + 1 device steps).
- Host computes E = exp(feat)*e^{-c} once (constant per-step log-shift c)
  and ships it as 16-bit; the device never runs exp.
- Per chain step: one matmul [K=100 -> M=104, N=GC] in 16-bit (1 cyc/row)
  into fp32 PSUM, then evacuation back into the marching E tile in 16-bit:
  either a direct DVE multiply from PSUM, or (for columns >= XD) an
  Act-engine PSUM->SBUF 16-bit copy followed by an all-SBUF 2x DVE multiply.
- d/z rows (100:104) ride along in the same ops and are DMA'd out per tile.
- Host stitches chain scales with the telescoping z-row recursion and
  computes len=0 outputs exactly (they need no feats).
"""

import sys

sys.path.insert(0, "/opt/trn_rl_repo")

import numpy as np

B, T, C = 1024, 512, 50
NCORES = 8
NEG = -10000.0

CFG = dict(
    S=4,        # sub-chains per core
    G=1,        # column groups per sub-chain
    W=1,        # warmup steps
    RTILE=3,    # steps per ring tile
    RB=6,       # ring depth per stream
    PB=2,       # PSUM bufs per stream
    XD=328,     # direct-DVE columns per group (None => all direct)
    XP=328,     # first Pool-lane column: [XD:XP] DVE-sbuf, [XP:GC] Pool
    DT="f16",  # 16-bit dtype: "f16" | "bf16"
    SPONLY=1,   # steady-state loads issued on SP only
    ACTFREE=1,  # keep the Act SEQ free of DMA issue (copies start sooner)
)

CSHIFT = float(7 * np.log(2.0))

_cached = {}


def _derived(cfg):
    S, G, W = cfg["S"], cfg["G"], cfg["W"]
    chunk = 64 // S
    nstep = chunk + W + 1
    gc = 512 // G
    xd = cfg["XD"] if cfg["XD"] is not None else gc
    xd = min(xd, gc)
    return chunk, nstep, gc, xd


def _np_dt(cfg):
    if cfg["DT"] == "f16":
        return np.float16
    import ml_dtypes

    return ml_dtypes.bfloat16


def _build_program(cfg):
    import concourse.bacc as bacc
    import concourse.tile as tile
    from concourse import mybir

    S, G, RTILE, RB, PB = cfg["S"], cfg["G"], cfg["RTILE"], cfg["RB"], cfg["PB"]
    chunk, NSTEP, GC, XD = _derived(cfg)
    assert NSTEP % RTILE == 0, (NSTEP, RTILE)
    NTILE = NSTEP // RTILE
    NS = S * G  # streams
    GBLK = NSTEP * GC

    f32 = mybir.dt.float32
    f16 = mybir.dt.float16 if cfg["DT"] == "f16" else mybir.dt.bfloat16
    nc = bacc.Bacc("TRN2", target_bir_lowering=False, debug=False)

    feats = nc.dram_tensor("feats", [104, NS * GBLK], f16, kind="ExternalInput")
    lhsT_in = nc.dram_tensor("lhsT_in", [100, 104], f16, kind="ExternalInput")
    p0_in = nc.dram_tensor("p0_in", [100, S * 512], f16, kind="ExternalInput")
    dzout = nc.dram_tensor("dzout", [4, NS * GBLK], f16, kind="ExternalOutput")

    with tile.TileContext(nc) as tc:
        with tc.tile_pool(name="singles", bufs=1) as singles:
            ieng0 = nc.scalar if cfg.get("P0ACT") else nc.sync
            lhsT = singles.tile([100, 104], f16)
            ieng0.dma_start(out=lhsT[:, :], in_=lhsT_in[:, :])
            p0 = singles.tile([100, S * 512], f16)
            ieng0.dma_start(out=p0[:, :], in_=p0_in[:, :])

            rings = []
            pmains = []
            scratches = []
            import contextlib

            stack = contextlib.ExitStack()
            with stack:
                pmains_b = []
                for ss in range(NS):
                    rings.append(
                        stack.enter_context(
                            tc.tile_pool(name=f"ring{ss}", bufs=RB)
                        )
                    )
                    pmains.append(
                        stack.enter_context(
                            tc.tile_pool(name=f"pm{ss}", bufs=PB, space="PSUM")
                        )
                    )
                    if XD < GC and cfg.get("SPLITMM"):
                        pmains_b.append(
                            stack.enter_context(
                                tc.tile_pool(
                                    name=f"pmb{ss}", bufs=PB, space="PSUM"
                                )
                            )
                        )
                    if XD < GC:
                        scratches.append(
                            stack.enter_context(
                                tc.tile_pool(name=f"sc{ss}", bufs=3)
                            )
                        )

                CHUNKC = RTILE * GC
                tiles = [[] for _ in range(NS)]

                def load_tile(ss, k, eng=None):
                    t_ = rings[ss].tile(
                        [104, CHUNKC], f16, name=f"ring{ss}_t", tag=f"ring{ss}_t"
                    )
                    base = ss * GBLK + k * CHUNKC
                    (eng or nc.sync).dma_start(
                        out=t_[:, :], in_=feats[:, base : base + CHUNKC]
                    )
                    tiles[ss].append(t_)

                PF = min(cfg.get("PFCAP") or (RB - 1), RB - 1, NTILE)
                # Startup: land slice 0 of every stream first (alternating
                # HWDGE issuers), then the rest of tile 0. Later tiles are
                # prefetched inside the loop, staggered across steps, so the
                # serial HWDGE device isn't a startup convoy.
                issuers = (
                    [nc.sync, nc.sync]
                    if cfg.get("ACTFREE")
                    else [nc.sync, nc.scalar]
                )
                first = []
                for ss in range(NS):
                    t_ = rings[ss].tile(
                        [104, CHUNKC], f16, name=f"ring{ss}_t", tag=f"ring{ss}_t"
                    )
                    base = ss * GBLK
                    issuers[ss % 2].dma_start(
                        out=t_[:, 0:GC], in_=feats[:, base : base + GC]
                    )
                    first.append(t_)
                for ss in range(NS):
                    base = ss * GBLK
                    issuers[ss % 2].dma_start(
                        out=first[ss][:, GC:CHUNKC],
                        in_=feats[:, base + GC : base + CHUNKC],
                    )
                    tiles[ss].append(first[ss])
                if not cfg.get("STAG") and not cfg.get("LATE"):
                    for k in range(1, min(PF, NTILE)):
                        for ss in range(NS):
                            load_tile(ss, k, eng=issuers[ss % 2])

                XP = cfg.get("XP") or GC  # [XD:XP] DVE-sbuf, [XP:GC] Pool

                for i in range(NSTEP):
                    k, sl = divmod(i, RTILE)
                    deferred = []
                    for s in range(S):
                        for g in range(G):
                            ss = s * G + g
                            if cfg.get("STAG"):
                                # staggered prefetch: at most one tile per
                                # stream per step, offset to spread HWDGE load
                                if (
                                    i % 2 == ss % 2
                                    and len(tiles[ss]) < NTILE
                                    and len(tiles[ss]) <= k + PF
                                ):
                                    load_tile(ss, len(tiles[ss]))
                            elif cfg.get("LATE"):
                                # prefetch issued at slice 1 so step-i
                                # consumers never queue behind it on the
                                # DMA completion semaphore
                                if (
                                    sl == 1
                                    and len(tiles[ss]) < NTILE
                                    and len(tiles[ss]) <= k + PF
                                ):
                                    eng = (
                                        nc.sync
                                        if cfg.get("SPONLY")
                                        else issuers[ss % 2]
                                    )
                                    load_tile(ss, len(tiles[ss]), eng=eng)
                            elif (
                                sl == 0
                                and k + PF < NTILE
                                and k + PF >= len(tiles[ss])
                            ):
                                eng = (
                                    nc.sync
                                    if cfg.get("SPONLY")
                                    else issuers[ss % 2]
                                )
                                load_tile(ss, k + PF, eng=eng)
                            cur = tiles[ss][k]
                            if i == 0:
                                rhs = p0[:, s * 512 + g * GC : s * 512 + (g + 1) * GC]
                            else:
                                pk, psl = divmod(i - 1, RTILE)
                                rhs = tiles[ss][pk][
                                    0:100, psl * GC : psl * GC + GC
                                ]
                            efsl = cur[:, sl * GC : (sl + 1) * GC]
                            if XD < GC and cfg.get("SPLITMM"):
                                # independent PSUM tiles for the direct-DVE
                                # and Act-copied ranges: the conservative
                                # same-tile dep tracking otherwise puts the
                                # Act copy on the direct mult's critical path
                                ps = pmains[ss].tile(
                                    [104, XD], f32,
                                    name=f"ps{ss}", tag=f"ps{ss}",
                                )
                                psb = pmains_b[ss].tile(
                                    [104, GC - XD], f32,
                                    name=f"psb{ss}", tag=f"psb{ss}",
                                )
                                nc.tensor.matmul(
                                    ps[:, :], lhsT[:, :], rhs[:, 0:XD],
                                    start=True, stop=True,
                                )
                                nc.tensor.matmul(
                                    psb[:, :], lhsT[:, :], rhs[:, XD:GC],
                                    start=True, stop=True,
                                )
                                sc = scratches[ss].tile(
                                    [104, GC - XD], f16,
                                    name=f"sc{ss}", tag=f"sc{ss}",
                                )
                                nc.scalar.copy(sc[:, :], psb[:, :])
                                nc.vector.tensor_mul(
                                    efsl[:, 0:XD], ps[:, :], efsl[:, 0:XD]
                                )
                                if XP > XD:
                                    deferred.append((
                                        "dve", efsl[:, XD:XP],
                                        sc[:, 0 : XP - XD],
                                    ))
                                if XP < GC:
                                    deferred.append((
                                        "pool", efsl[:, XP:GC],
                                        sc[:, XP - XD : GC - XD],
                                    ))
                                continue
                            ps = pmains[ss].tile(
                                [104, GC], f32, name=f"ps{ss}", tag=f"ps{ss}"
                            )
                            nc.tensor.matmul(
                                ps[:, :], lhsT[:, :], rhs, start=True, stop=True
                            )
                            if XD < GC:
                                sc = scratches[ss].tile(
                                    [104, GC - XD], f16,
                                    name=f"sc{ss}", tag=f"sc{ss}",
                                )
                                if cfg.get("MULTFIRST"):
                                    nc.vector.tensor_mul(
                                        efsl[:, 0:XD], ps[:, 0:XD],
                                        efsl[:, 0:XD],
                                    )
                                    nc.scalar.copy(sc[:, :], ps[:, XD:GC])
                                else:
                                    nc.scalar.copy(sc[:, :], ps[:, XD:GC])
                                    nc.vector.tensor_mul(
                                        efsl[:, 0:XD], ps[:, 0:XD],
                                        efsl[:, 0:XD],
                                    )
                                # defer scratch-sourced mults behind all
                                # direct mults to avoid head-of-line blocks
                                if XP > XD:
                                    deferred.append((
                                        "dve", efsl[:, XD:XP],
                                        sc[:, 0 : XP - XD],
                                    ))
                                if XP < GC:
                                    deferred.append((
                                        "pool", efsl[:, XP:GC],
                                        sc[:, XP - XD : GC - XD],
                                    ))
                            else:
                                nc.vector.tensor_mul(efsl, ps[:, :], efsl)
                    for kind, dst, src in deferred:
                        eng = nc.vector if kind == "dve" else nc.gpsimd
                        eng.tensor_mul(dst, src, dst)
                    if sl == RTILE - 1 and not cfg.get("OUTEND"):
                        for ss in range(NS):
                            base = ss * GBLK + k * CHUNKC
                            nc.sync.dma_start(
                                out=dzout[:, base : base + CHUNKC],
                                in_=tiles[ss][k][100:104, :],
                            )
                if cfg.get("OUTEND"):
                    # no ring reuse (RB >= NTILE): one dz out-DMA per tile,
                    # all issued at the end
                    for ss in range(NS):
                        for k in range(NTILE):
                            base = ss * GBLK + k * CHUNKC
                            nc.sync.dma_start(
                                out=dzout[:, base : base + CHUNKC],
                                in_=tiles[ss][k][100:104, :],
                            )

    nc.compile()
    return nc


def _get_program(cfg=None):
    key = tuple(sorted((k, v) for k, v in (cfg or CFG).items()))
    if key not in _cached:
        _cached[key] = _build_program(cfg or CFG)
    return _cached[key]


def _pack_feats_core(EF, c, cfg):
    """EF: [B, T, C] float32 = exp(feats)*e^{-CSHIFT}. Returns the packed
    [104, NS*GBLK] 16-bit array for core c."""
    S, G = cfg["S"], cfg["G"]
    chunk, NSTEP, GC, _ = _derived(cfg)
    dt = _np_dt(cfg)
    GBLK = NSTEP * GC
    econst = np.float32(np.exp(-CSHIFT))
    out = np.full((104, S * G * GBLK), econst, dtype=np.float32)
    for s in range(S):
        j = c * S + s
        start = 0 if j == 0 else chunk * j - cfg["W"]
        ts = start + np.arange(NSTEP)
        valid = ts < T
        f = EF[:, np.minimum(ts, T - 1), :]
        if not valid.all():
            f = np.where(valid[None, :, None], f, econst)
        x = (
            f.reshape(2, G, GC, NSTEP, C)
            .transpose(1, 0, 4, 3, 2)  # [G, 2, C, NSTEP, GC]
            .reshape(G, 100, GBLK)
        )
        for g in range(G):
            ss = s * G + g
            out[:100, ss * GBLK : (ss + 1) * GBLK] = x[g]
    return np.ascontiguousarray(out.astype(dt))


def _host_inputs(lstm_feats, transitions, cfg):
    S, G = cfg["S"], cfg["G"]
    chunk, NSTEP, GC, _ = _derived(cfg)
    dt = _np_dt(cfg)
    feats = np.asarray(lstm_feats, dtype=np.float32)
    trans = np.asarray(transitions, dtype=np.float32)

    EF = np.exp(feats) * np.float32(np.exp(-CSHIFT))

    A = np.exp(trans)
    lhsT = np.zeros((100, 104), np.float32)
    lhsT[0:50, 0:50] = A.T
    lhsT[50:100, 50:100] = A.T
    lhsT[0:50, 100] = A[49, :]     # d row, elem half 0 (EST = exp(trans[49,:]))
    lhsT[50:100, 101] = A[49, :]   # d row, half 1
    lhsT[0:50, 102] = 1.0          # z row, half 0
    lhsT[50:100, 103] = 1.0        # z row, half 1
    lhsT16 = np.ascontiguousarray(lhsT.astype(dt))

    in_maps = []
    for c in range(NCORES):
        p0 = np.zeros((100, S * 512), np.float32)
        for s in range(S):
            j = c * S + s
            if j == 0:
                p0[48, s * 512 : (s + 1) * 512] = 1.0
                p0[98, s * 512 : (s + 1) * 512] = 1.0
            else:
                p0[:, s * 512 : (s + 1) * 512] = 1.0 / C
        in_maps.append(
            {
                "feats": _pack_feats_core(EF, c, cfg),
                "lhsT_in": lhsT16,
                "p0_in": np.ascontiguousarray(p0.astype(dt)),
            }
        )
    return in_maps


def _assemble(results, lens_np, trans, cfg):
    S, G, W = cfg["S"], cfg["G"], cfg["W"]
    chunk, NSTEP, GC, _ = _derived(cfg)
    GBLK = NSTEP * GC
    NCH = NCORES * S

    bidx = np.arange(B)
    half = bidx // 512
    idx = bidx % 512
    grp = idx // GC
    xcol = idx % GC
    ii = np.arange(NSTEP)

    dmat = np.zeros((NCH, NSTEP, B))
    zmat = np.zeros((NCH, NSTEP, B))
    for c in range(NCORES):
        dz = np.asarray(results[c]["dzout"], dtype=np.float64)
        for s in range(S):
            j = c * S + s
            ss0 = s * G
            cols = (ss0 + grp)[None, :] * GBLK + ii[:, None] * GC + xcol[None, :]
            dmat[j] = dz[half[None, :], cols]
            zmat[j] = dz[2 + half[None, :], cols]

    logsig = np.zeros((NCH, B))
    for j in range(1, NCH):
        i_prev = chunk if j == 1 else chunk + W
        lam_prev = np.log(zmat[j - 1, i_prev]) + CSHIFT * (i_prev + 1)
        lam_cur = np.log(zmat[j, W]) + CSHIFT * (W + 1)
        logsig[j] = logsig[j - 1] + lam_prev - lam_cur

    owner = np.minimum(lens_np // chunk, NCH - 1).astype(np.int64)
    dev_i = np.where(owner == 0, lens_np, lens_np - (chunk * owner - W))
    out = np.zeros(B)
    for j in range(NCH):
        m = owner == j
        if m.any():
            iim = dev_i[m]
            out[m] = np.log(dmat[j, iim, m]) + CSHIFT * (iim + 1) + logsig[j, m]
    out[lens_np == 0] = np.float64(trans[49, 48])
    return out.astype(np.float32)


def kernel(lstm_feats, lens, transitions):
    from concourse.bass_utils import run_bass_kernel_spmd

    cfg = CFG
    lens_np = np.asarray(lens).astype(np.int64)
    trans = np.asarray(transitions, dtype=np.float32)

    nc = _get_program(cfg)
    in_maps = _host_inputs(lstm_feats, trans, cfg)
    res = run_bass_kernel_spmd(nc, in_maps, list(range(NCORES)))
    global _last_exec_ns
    _last_exec_ns = res.exec_time_ns

    return _assemble(res.results, lens_np, trans, cfg)
